# revision 1
# baseline (speedup 1.0000x reference)
"""CrossAttention1D Trainium2 kernel.

Problem: B=4, C=1024, L=2048, H=16 heads (D=64). LKV == LQ so the
reference's linear interpolation is the identity and is skipped.

Sharding (8 cores): data-parallel over batch (4) x tensor-parallel over
heads (2 halves of 8 heads). Core c handles batch c//2, heads
(c%2)*8 .. (c%2)*8+8. Each core computes its half of Q/K/V projections
(512 of 1024 channels), attention for its 8 heads, and a partial output
projection Wo[:, shard] @ O (+ residual/bias on even cores). The host
sums the two partials per batch.

Device dataflow per core (all matmuls bf16 with fp32 PSUM accumulate):
  Q  = WqT.T @ query      [512, 2048]  (channels on partitions)
  K  = WkT.T @ context    [512, 2048]
  VT = context.T @ WvT    [2048, 512]  (positions on partitions), stored
       interleaved with a ones column per head -> [2048, 8*65] so the AV
       matmul's 65th row accumulates the softmax denominator for free.
  Per head pair (heads 2t at partitions 0:64, 2t+1 at 64:128 feed
  row-group-paired k=64 matmuls that run concurrently on the PE):
    S^T[j,i] = K_h.T @ Q_h    (per 128-row j-tile, PSUM)
    P^T = exp(S^T / 8)        (ScalarE, PSUM->SBUF bf16)
    O_ext[d(+sum),i] += V_ext^T[jt].T @ P^T[jt]   (m=65, accumulated)
  O = O_ext[0:64] * recip(O_ext[64]) broadcast     -> bf16
  out = WoT.T @ O + resid                          -> fp32 partial
"""

import json

import numpy as np
import ml_dtypes

import concourse.bass as bass
import concourse.mybir as mybir
import concourse.tile as tile
from concourse.bass_utils import run_bass_kernel_spmd

BF16 = ml_dtypes.bfloat16

B, C, L, H, D = 4, 1024, 2048, 16, 64
CS = C // 2          # channel shard per core (512)
HPC = H // 2         # heads per core (8)
NCORES = 8
SCALE = 1.0 / np.sqrt(D)  # 0.125

_DT = mybir.dt

_MAX_WAITS = 1


def _split_drain_waits(nc):
    """Hoist excess per-instruction sync-waits onto preceding NoOps.

    This toolchain's walrus codegen rejects instructions carrying more
    than one sync wait ("Too many sync wait commands"). Hoisting a wait
    onto a NoOp immediately before the instruction on the same engine is
    semantics-preserving (engines execute their stream in order).
    """
    j = json.loads(nc.to_json_bytes())
    n_hoisted = 0
    for fn in j["functions"]:
        for bb in fn["blocks"]:
            out = []
            for inst in bb["instructions"]:
                si = inst.get("sync_info")
                ow = (si or {}).get("on_wait") or []
                if len(ow) > _MAX_WAITS:
                    n_hoisted += 1
                    for i, w in enumerate(ow[: -_MAX_WAITS]):
                        out.append(
                            {
                                "engine": inst["engine"],
                                "ins": [],
                                "outs": [],
                                "name": f"{inst['name']}_hw{i}",
                                "opcode": "NoOp",
                                "debug": inst.get("debug"),
                                "sync_info": {"on_update": [], "on_wait": [w]},
                            }
                        )
                    si["on_wait"] = ow[-_MAX_WAITS:]
                out.append(inst)
            bb["instructions"] = out
    patched = json.dumps(j).encode()
    nc.to_json_bytes = lambda: patched
    return nc


def _build_nc():
    nc = bass.Bass()
    dt = _DT
    bf = dt.bfloat16
    f32 = dt.float32

    q_d = nc.declare_dram_parameter("q_bf", [C, L], bf, isOutput=False)
    ctx_d = nc.declare_dram_parameter("ctx_bf", [C, L], bf, isOutput=False)
    wq_d = nc.declare_dram_parameter("wqT", [C, CS], bf, isOutput=False)
    wk_d = nc.declare_dram_parameter("wkT", [C, CS], bf, isOutput=False)
    wv_d = nc.declare_dram_parameter("wvT", [C, CS], bf, isOutput=False)
    wo_d = nc.declare_dram_parameter("woT", [CS, C], bf, isOutput=False)
    res_d = nc.declare_dram_parameter("resid", [C, L], f32, isOutput=False)
    out_d = nc.declare_dram_parameter("out", [C, L], f32, isOutput=True)

    KT = C // 128        # 8 contraction tiles for projections
    CT = CS // 128       # 4 channel tiles of the shard
    LT = L // 512        # 4 L-tiles of 512
    JT = L // 128        # 16 j-tiles of 128

    with tile.TileContext(nc) as tc:
        with (
            tc.tile_pool(name="const", bufs=1) as cp,
            tc.tile_pool(name="pwork", bufs=3) as pwork,
            tc.tile_pool(name="norm", bufs=2) as normp,
            tc.tile_pool(name="io", bufs=3) as iop,
            tc.tile_pool(name="psum", bufs=3, space="PSUM") as psp,
        ):
            # ---- resident SBUF slabs
            q_sb = cp.tile([128, KT, L], bf)       # query, c_in on partitions
            c_sb = cp.tile([128, KT, L], bf)       # context
            wq_sb = cp.tile([128, KT, CS], bf)
            wk_sb = cp.tile([128, KT, CS], bf)
            wv_sb = cp.tile([128, KT, CS], bf)
            wo_sb = cp.tile([128, CT, C], bf)
            Q_sb = cp.tile([128, CT, L], bf)       # projected Q (bf16)
            K_sb = cp.tile([128, CT, L], bf)
            V_sb = cp.tile([128, JT, HPC * (D + 1)], bf)  # V^T + ones cols
            O_sb = cp.tile([128, CT, L], bf)       # normalized attn output

            qr = q_d.rearrange("(k p) m -> p k m", p=128)
            cr = ctx_d.rearrange("(k p) m -> p k m", p=128)
            wqr = wq_d.rearrange("(k p) m -> p k m", p=128)
            wkr = wk_d.rearrange("(k p) m -> p k m", p=128)
            wvr = wv_d.rearrange("(k p) m -> p k m", p=128)
            wor = wo_d.rearrange("(k p) m -> p k m", p=128)
            for kt in range(KT):
                nc.sync.dma_start(q_sb[:, kt, :], qr[:, kt, :])
                nc.sync.dma_start(c_sb[:, kt, :], cr[:, kt, :])
                nc.sync.dma_start(wq_sb[:, kt, :], wqr[:, kt, :])
                nc.sync.dma_start(wk_sb[:, kt, :], wkr[:, kt, :])
                nc.sync.dma_start(wv_sb[:, kt, :], wvr[:, kt, :])
            for kt in range(CT):
                nc.sync.dma_start(wo_sb[:, kt, :], wor[:, kt, :])

            # ones columns for the AV denominator rows
            v_view = V_sb.rearrange("p j (h e) -> p j h e", e=D + 1)
            for jt in range(JT):
                nc.vector.memset(v_view[:, jt, :, D : D + 1], 1.0)
            ones_sb = cp.tile([1, 64], f32)
            nc.vector.memset(ones_sb, 1.0)

            # ---- projections: Q, K (c_out on partitions), V^T (j on partitions)
            for ct in range(CT):
                for lt in range(LT):
                    ls = slice(lt * 512, (lt + 1) * 512)
                    pq = psp.tile([128, 512], f32, tag="s")
                    for kt in range(KT):
                        nc.tensor.matmul(
                            pq,
                            lhsT=wq_sb[:, kt, ct * 128 : (ct + 1) * 128],
                            rhs=q_sb[:, kt, ls],
                            start=(kt == 0),
                            stop=(kt == KT - 1),
                        )
                    nc.vector.tensor_copy(Q_sb[:, ct, ls], pq)
                    pk = psp.tile([128, 512], f32, tag="s")
                    for kt in range(KT):
                        nc.tensor.matmul(
                            pk,
                            lhsT=wk_sb[:, kt, ct * 128 : (ct + 1) * 128],
                            rhs=c_sb[:, kt, ls],
                            start=(kt == 0),
                            stop=(kt == KT - 1),
                        )
                    nc.vector.tensor_copy(K_sb[:, ct, ls], pk)
            for jt in range(JT):
                pv = psp.tile([128, 512], f32, tag="s")
                for kt in range(KT):
                    nc.tensor.matmul(
                        pv,
                        lhsT=c_sb[:, kt, jt * 128 : (jt + 1) * 128],
                        rhs=wv_sb[:, kt, :],
                        start=(kt == 0),
                        stop=(kt == KT - 1),
                    )
                nc.vector.tensor_copy(
                    v_view[:, jt, :, 0:D],
                    pv.rearrange("p (h d) -> p h d", d=D),
                )

            # ---- attention + output projection, per i-tile epoch.
            # One [128,1024] PSUM tile holds both heads' S^T per j-tile so
            # a single exp covers the pair; the QK->AV pipeline runs 2 j
            # iterations deep, and Wo-projection matmuls for the previous
            # i-tile are interleaved into the loop as PE filler so the PE
            # never idles waiting on ACT (HAM throttle avoidance).
            Exp = mybir.ActivationFunctionType.Exp
            DEPTH = 2

            def emit_wo(mt, it_prev):
                psl = slice(it_prev * 512, (it_prev + 1) * 512)
                po = psp.tile([128, 512], f32, tag="s")
                for kt in range(CT):
                    nc.tensor.matmul(
                        po,
                        lhsT=wo_sb[:, kt, mt * 128 : (mt + 1) * 128],
                        rhs=O_sb[:, kt, psl],
                        start=(kt == 0),
                        stop=(kt == CT - 1),
                    )
                rt = iop.tile([128, 512], f32, tag="res")
                nc.sync.dma_start(rt, res_d[mt * 128 : (mt + 1) * 128, psl])
                ot = iop.tile([128, 512], f32, tag="out")
                nc.vector.tensor_add(ot, po, rt)
                nc.sync.dma_start(out_d[mt * 128 : (mt + 1) * 128, psl], ot)

            for it in range(LT):
                isl = slice(it * 512, (it + 1) * 512)
                wo_queue = list(range(C // 128)) if it > 0 else []
                for tp in range(CT):  # head pair (2*tp, 2*tp+1)
                    pOa = psp.tile([D + 1, 512], f32, tag="acc", bufs=2)
                    pOb = psp.tile([D + 1, 512], f32, tag="acc", bufs=2)
                    pend = []
                    for jt in range(JT + DEPTH):
                        if jt < JT:
                            js = slice(jt * 128, (jt + 1) * 128)
                            pS = psp.tile([128, 1024], f32, tag="s")
                            nc.tensor.matmul(
                                pS[:, 0:512],
                                lhsT=K_sb[0:64, tp, js],
                                rhs=Q_sb[0:64, tp, isl],
                                start=True,
                                stop=True,
                            )
                            nc.tensor.matmul(
                                pS[:, 512:1024],
                                lhsT=K_sb[64:128, tp, js],
                                rhs=Q_sb[64:128, tp, isl],
                                start=True,
                                stop=True,
                            )
                            Pab = pwork.tile([128, 1024], bf, tag="p")
                            nc.scalar.activation(Pab, pS, Exp, scale=SCALE)
                            pend.append((Pab, jt))
                        if len(pend) > (DEPTH if jt < JT else 0):
                            Pab, qjt = pend.pop(0)
                            ha, hb = 2 * tp, 2 * tp + 1
                            nc.tensor.matmul(
                                pOa,
                                lhsT=V_sb[:, qjt, ha * 65 : ha * 65 + 65],
                                rhs=Pab[:, 0:512],
                                start=(qjt == 0),
                                stop=(qjt == JT - 1),
                            )
                            nc.tensor.matmul(
                                pOb,
                                lhsT=V_sb[:, qjt, hb * 65 : hb * 65 + 65],
                                rhs=Pab[:, 512:1024],
                                start=(qjt == 0),
                                stop=(qjt == JT - 1),
                            )
                        if jt % 8 == 7 and wo_queue:
                            emit_wo(wo_queue.pop(0), it - 1)

                    # normalize both heads of the pair
                    for hh, pO in ((2 * tp, pOa), (2 * tp + 1, pOb)):
                        row = normp.tile([1, 512], f32, tag="row")
                        nc.vector.tensor_copy(row, pO[D : D + 1, :])
                        rec = normp.tile([1, 512], f32, tag="rec")
                        nc.vector.reciprocal(rec, row)
                        rb_ps = psp.tile([64, 512], f32, tag="s")
                        nc.tensor.matmul(
                            rb_ps, lhsT=ones_sb, rhs=rec, start=True, stop=True
                        )
                        rbc = normp.tile([64, 512], f32, tag="rbc")
                        nc.vector.tensor_copy(rbc, rb_ps)
                        otmp = normp.tile([64, 512], bf, tag="otmp")
                        nc.vector.tensor_mul(otmp, pO[0:D, :], rbc)
                        poff = (hh % 2) * 64
                        nc.sync.dma_start(
                            O_sb[poff : poff + 64, hh // 2, isl], otmp
                        )
                for mt in wo_queue:
                    emit_wo(mt, it - 1)
            for mt in range(C // 128):
                emit_wo(mt, LT - 1)
    return nc


_NC = None


def _get_nc():
    global _NC
    if _NC is None:
        _NC = _split_drain_waits(_build_nc())
    return _NC


def _make_in_maps(query, context, Wq, Wk, Wv, Wo, bo):
    zeros_res = np.zeros((C, L), np.float32)
    in_maps = []
    for c in range(NCORES):
        b, hf = c // 2, c % 2
        rows = slice(hf * CS, (hf + 1) * CS)
        in_maps.append(
            {
                "q_bf": query[b].astype(BF16),
                "ctx_bf": context[b].astype(BF16),
                "wqT": np.ascontiguousarray(Wq[rows].T).astype(BF16),
                "wkT": np.ascontiguousarray(Wk[rows].T).astype(BF16),
                "wvT": np.ascontiguousarray(Wv[rows].T).astype(BF16),
                "woT": np.ascontiguousarray(Wo[:, rows].T).astype(BF16),
                "resid": (query[b] + bo[:, None]).astype(np.float32)
                if hf == 0
                else zeros_res,
            }
        )
    return in_maps


def _gather(results):
    out = np.empty((B, C, L), np.float32)
    for b in range(B):
        out[b] = results[2 * b]["out"] + results[2 * b + 1]["out"]
    return out


def kernel(query, context, Wq, Wk, Wv, Wo, bo, heads):
    query = np.asarray(query, dtype=np.float32)
    context = np.asarray(context, dtype=np.float32)
    Wq = np.asarray(Wq, dtype=np.float32)
    Wk = np.asarray(Wk, dtype=np.float32)
    Wv = np.asarray(Wv, dtype=np.float32)
    Wo = np.asarray(Wo, dtype=np.float32)
    bo = np.asarray(bo, dtype=np.float32)
    assert int(heads) == H
    assert query.shape == (B, C, L) and context.shape == (B, C, L)

    nc = _get_nc()
    in_maps = _make_in_maps(query, context, Wq, Wk, Wv, Wo, bo)
    res = run_bass_kernel_spmd(nc, in_maps, list(range(NCORES))).results
    return _gather(res)



# revision 3
# speedup vs baseline: 1.2014x; 1.2014x over previous
"""CrossAttention1D Trainium2 kernel (fp8 DoubleRow edition).

Problem: B=4, C=1024, L=2048, H=16 heads (D=64). LKV == LQ so the
reference's linear interpolation is the identity and is skipped.

Sharding (8 cores): data-parallel over batch (4) x tensor-parallel over
heads (2 halves of 8 heads). Core c handles batch c//2, heads
(c%2)*8 .. (c%2)*8+8. Each core computes its half of Q/K/V projections
(512 of 1024 channels), attention for its 8 heads, and a partial output
projection Wo[:, shard] @ O (+ residual/bias on even cores). The host
sums the two partials per batch.

Device dataflow per core:
  All projections + AV + Wo run as fp8e4m3 DoubleRow matmuls (2 k-tiles
  of 128 packed per instruction, 2x PE throughput); QK^T stays bf16 with
  k=64 row-group pairing (both heads of a pair concurrently on the PE).
  Softmax uses exp(S/8 - 3) (shift-invariant; keeps P inside fp8 range)
  written by ACT directly to fp8, with a ones-column in the V stationary
  accumulating the denominator row during AV. O is normalized by
  reciprocal_approx_fast + PE broadcast, stored fp8 for the Wo DoubleRow.
"""

import json

import numpy as np
import ml_dtypes

import concourse.bass as bass
import concourse.mybir as mybir
import concourse.tile as tile
from concourse.bass_utils import run_bass_kernel_spmd

BF16 = ml_dtypes.bfloat16
F8 = ml_dtypes.float8_e4m3

B, C, L, H, D = 4, 1024, 2048, 16, 64
CS = C // 2          # channel shard per core (512)
HPC = H // 2         # heads per core (8)
NCORES = 8
SCALE = 1.0 / np.sqrt(D)  # 0.125
EXP_BIAS = -3.0      # exp(S*SCALE + EXP_BIAS): softmax-invariant shift
VP = 80              # per-head stride in V_sb (65 used, 16B-aligned)

_DT = mybir.dt

_MAX_WAITS = 1


def _split_drain_waits(nc):
    """Hoist excess per-instruction sync-waits onto preceding NoOps.

    This toolchain's walrus codegen rejects instructions carrying more
    than one sync wait ("Too many sync wait commands"). Hoisting a wait
    onto a NoOp immediately before the instruction on the same engine is
    semantics-preserving (engines execute their stream in order).
    """
    j = json.loads(nc.to_json_bytes())
    n_hoisted = 0
    for fn in j["functions"]:
        for bb in fn["blocks"]:
            out = []
            for inst in bb["instructions"]:
                si = inst.get("sync_info")
                ow = (si or {}).get("on_wait") or []
                if len(ow) > _MAX_WAITS:
                    n_hoisted += 1
                    for i, w in enumerate(ow[: -_MAX_WAITS]):
                        out.append(
                            {
                                "engine": inst["engine"],
                                "ins": [],
                                "outs": [],
                                "name": f"{inst['name']}_hw{i}",
                                "opcode": "NoOp",
                                "debug": inst.get("debug"),
                                "sync_info": {"on_update": [], "on_wait": [w]},
                            }
                        )
                    si["on_wait"] = ow[-_MAX_WAITS:]
                out.append(inst)
            bb["instructions"] = out
    patched = json.dumps(j).encode()
    nc.to_json_bytes = lambda: patched
    return nc


def _build_nc():
    nc = bass.Bass()
    dt = _DT
    bf = dt.bfloat16
    f8 = dt.float8e4
    f32 = dt.float32
    DR = mybir.MatmulPerfMode.DoubleRow

    q_d = nc.declare_dram_parameter("q8", [C, L], f8, isOutput=False)
    ctx_d = nc.declare_dram_parameter("ctx8", [C, L], f8, isOutput=False)
    wq_d = nc.declare_dram_parameter("wqT", [C, CS], f8, isOutput=False)
    wk_d = nc.declare_dram_parameter("wkT", [C, CS], f8, isOutput=False)
    wv_d = nc.declare_dram_parameter("wvT", [C, CS], f8, isOutput=False)
    wo_d = nc.declare_dram_parameter("woT", [CS, C], f8, isOutput=False)
    res_d = nc.declare_dram_parameter("resid", [C, L], f32, isOutput=False)
    out_d = nc.declare_dram_parameter("out", [C, L], f32, isOutput=True)

    KT = C // 128        # 8 contraction tiles for projections
    KP = KT // 2         # 4 DoubleRow k-pairs
    CT = CS // 128       # 4 channel tiles of the shard
    CP = CT // 2         # 2 DoubleRow k-pairs for Wo
    LT = L // 512        # 4 L-tiles of 512
    JT = L // 128        # 16 j-tiles of 128
    JP = JT // 2         # 8 j-pairs (DoubleRow AV)

    with tile.TileContext(nc) as tc:
        with (
            tc.tile_pool(name="const", bufs=1) as cp,
            tc.tile_pool(name="pwork", bufs=3) as pwork,
            tc.tile_pool(name="norm", bufs=2) as normp,
            tc.tile_pool(name="io", bufs=3) as iop,
            tc.tile_pool(name="psum", bufs=3, space="PSUM") as psp,
        ):
            # ---- resident SBUF slabs
            q_sb = cp.tile([128, KT, L], f8)       # query, c_in on partitions
            c_sb = cp.tile([128, KT, L], f8)       # context
            wq_sb = cp.tile([128, KT, CS], f8)
            wk_sb = cp.tile([128, KT, CS], f8)
            wv_sb = cp.tile([128, KT, CS], f8)
            wo_sb = cp.tile([128, CT, C], f8)
            Q_sb = cp.tile([128, CT, L], bf)       # projected Q (bf16)
            K_sb = cp.tile([128, CT, L], bf)
            V_sb = cp.tile([128, JT, HPC * VP], f8)  # V^T + ones cols, padded
            O_sb = cp.tile([128, CT, L], f8)       # normalized attn output

            qr = q_d.rearrange("(k p) m -> p k m", p=128)
            cr = ctx_d.rearrange("(k p) m -> p k m", p=128)
            wqr = wq_d.rearrange("(k p) m -> p k m", p=128)
            wkr = wk_d.rearrange("(k p) m -> p k m", p=128)
            wvr = wv_d.rearrange("(k p) m -> p k m", p=128)
            wor = wo_d.rearrange("(k p) m -> p k m", p=128)
            for kt in range(KT):
                nc.sync.dma_start(q_sb[:, kt, :], qr[:, kt, :])
                nc.sync.dma_start(c_sb[:, kt, :], cr[:, kt, :])
                nc.sync.dma_start(wq_sb[:, kt, :], wqr[:, kt, :])
                nc.sync.dma_start(wk_sb[:, kt, :], wkr[:, kt, :])
                nc.sync.dma_start(wv_sb[:, kt, :], wvr[:, kt, :])
            for kt in range(CT):
                nc.sync.dma_start(wo_sb[:, kt, :], wor[:, kt, :])

            # ones columns for the AV denominator rows
            v_view = V_sb.rearrange("p j (h e) -> p j h e", e=VP)
            for jt in range(JT):
                nc.vector.memset(v_view[:, jt, :, D : D + 1], 1.0)
            ones_sb = cp.tile([1, 64], f32)
            nc.vector.memset(ones_sb, 1.0)
            bias_sb = cp.tile([128, 1], f32)
            nc.vector.memset(bias_sb, EXP_BIAS)

            # ---- projections (fp8 DoubleRow, contraction 256/matmul)
            # K first, then V, then Q i-tile 0: attention can start early.
            def emit_proj_qk(dst, w_sb, src, ct, lt):
                ls = slice(lt * 512, (lt + 1) * 512)
                p = psp.tile([128, 512], f32, tag="s")
                for kp in range(KP):
                    nc.tensor.matmul(
                        p,
                        lhsT=w_sb[:, 2 * kp : 2 * kp + 2, ct * 128 : (ct + 1) * 128],
                        rhs=src[:, 2 * kp : 2 * kp + 2, ls],
                        start=(kp == 0),
                        stop=(kp == KP - 1),
                        perf_mode=DR,
                    )
                nc.vector.tensor_copy(dst[:, ct, ls], p)

            def emit_proj_v(jt):
                pv = psp.tile([128, 512], f32, tag="s")
                for kp in range(KP):
                    nc.tensor.matmul(
                        pv,
                        lhsT=c_sb[:, 2 * kp : 2 * kp + 2, jt * 128 : (jt + 1) * 128],
                        rhs=wv_sb[:, 2 * kp : 2 * kp + 2, :],
                        start=(kp == 0),
                        stop=(kp == KP - 1),
                        perf_mode=DR,
                    )
                nc.vector.tensor_copy(
                    v_view[:, jt, :, 0:D],
                    pv.rearrange("p (h d) -> p h d", d=D),
                )

            for ct in range(CT):
                for lt in range(LT):
                    emit_proj_qk(K_sb, wk_sb, c_sb, ct, lt)
            for jt in range(JT):
                emit_proj_v(jt)
            for ct in range(CT):
                emit_proj_qk(Q_sb, wq_sb, q_sb, ct, 0)

            # ---- attention + output projection, per i-tile epoch.
            # QK (bf16, k=64 row-group paired) -> exp to fp8 -> AV DoubleRow.
            # Wo-projection matmuls for the previous i-tile and the
            # remaining Q projections are interleaved as PE filler.
            Exp = mybir.ActivationFunctionType.Exp
            DEPTH = 2

            def emit_wo(mt, it_prev):
                psl = slice(it_prev * 512, (it_prev + 1) * 512)
                po = psp.tile([128, 512], f32, tag="s")
                for kp in range(CP):
                    nc.tensor.matmul(
                        po,
                        lhsT=wo_sb[:, 2 * kp : 2 * kp + 2, mt * 128 : (mt + 1) * 128],
                        rhs=O_sb[:, 2 * kp : 2 * kp + 2, psl],
                        start=(kp == 0),
                        stop=(kp == CP - 1),
                        perf_mode=DR,
                    )
                rt = iop.tile([128, 512], f32, tag="res")
                nc.sync.dma_start(rt, res_d[mt * 128 : (mt + 1) * 128, psl])
                ot = iop.tile([128, 512], f32, tag="out")
                nc.vector.tensor_add(ot, po, rt)
                nc.sync.dma_start(out_d[mt * 128 : (mt + 1) * 128, psl], ot)

            for it in range(LT):
                isl = slice(it * 512, (it + 1) * 512)
                filler = []
                if it > 0:
                    # Q projection for this epoch's successor is already done
                    # (lt=it emitted below); Wo for previous epoch.
                    filler = [("wo", mt) for mt in range(C // 128)]
                if it < LT - 1:
                    filler += [("q", ct) for ct in range(CT)]
                for tp in range(CT):  # head pair (2*tp, 2*tp+1)
                    pOa = psp.tile([D + 1, 512], f32, tag="acc", bufs=2)
                    pOb = psp.tile([D + 1, 512], f32, tag="acc", bufs=2)
                    pend = []
                    for jp in range(JP + DEPTH):
                        if jp < JP:
                            Pab = pwork.tile([128, 2, 1024], f8, tag="p")
                            for t in range(2):
                                jt = 2 * jp + t
                                js = slice(jt * 128, (jt + 1) * 128)
                                pS = psp.tile([128, 1024], f32, tag="s")
                                nc.tensor.matmul(
                                    pS[:, 0:512],
                                    lhsT=K_sb[0:64, tp, js],
                                    rhs=Q_sb[0:64, tp, isl],
                                    start=True,
                                    stop=True,
                                )
                                nc.tensor.matmul(
                                    pS[:, 512:1024],
                                    lhsT=K_sb[64:128, tp, js],
                                    rhs=Q_sb[64:128, tp, isl],
                                    start=True,
                                    stop=True,
                                )
                                nc.scalar.activation(
                                    Pab[:, t, :], pS, Exp,
                                    bias=bias_sb[:, :], scale=SCALE,
                                )
                            pend.append((Pab, jp))
                        if len(pend) > (DEPTH if jp < JP else 0):
                            Pab_r, qjp = pend.pop(0)
                            ha, hb = 2 * tp, 2 * tp + 1
                            nc.tensor.matmul(
                                pOa,
                                lhsT=V_sb[:, 2 * qjp : 2 * qjp + 2,
                                          ha * VP : ha * VP + D + 1],
                                rhs=Pab_r[:, 0:2, 0:512],
                                start=(qjp == 0),
                                stop=(qjp == JP - 1),
                                perf_mode=DR,
                            )
                            nc.tensor.matmul(
                                pOb,
                                lhsT=V_sb[:, 2 * qjp : 2 * qjp + 2,
                                          hb * VP : hb * VP + D + 1],
                                rhs=Pab_r[:, 0:2, 512:1024],
                                start=(qjp == 0),
                                stop=(qjp == JP - 1),
                                perf_mode=DR,
                            )
                        if jp % 2 == 1 and filler:
                            kind, arg = filler.pop(0)
                            if kind == "wo":
                                emit_wo(arg, it - 1)
                            else:
                                emit_proj_qk(Q_sb, wq_sb, q_sb, arg, it + 1)

                    # normalize both heads of the pair
                    for hh, pO in ((2 * tp, pOa), (2 * tp + 1, pOb)):
                        rec = normp.tile([1, 512], f32, tag="rec")
                        nc.vector.reciprocal(rec, pO[D : D + 1, :])
                        rb_ps = psp.tile([64, 512], f32, tag="s")
                        nc.tensor.matmul(
                            rb_ps, lhsT=ones_sb, rhs=rec, start=True, stop=True
                        )
                        rbc = normp.tile([64, 512], f32, tag="rbc")
                        nc.vector.tensor_copy(rbc, rb_ps)
                        otmp = normp.tile([64, 512], f8, tag="otmp")
                        nc.vector.tensor_mul(otmp, pO[0:D, :], rbc)
                        poff = (hh % 2) * 64
                        nc.sync.dma_start(
                            O_sb[poff : poff + 64, hh // 2, isl], otmp
                        )
                for kind, arg in filler:
                    if kind == "wo":
                        emit_wo(arg, it - 1)
                    else:
                        emit_proj_qk(Q_sb, wq_sb, q_sb, arg, it + 1)
            for mt in range(C // 128):
                emit_wo(mt, LT - 1)
    return nc


_NC = None


def _get_nc():
    global _NC
    if _NC is None:
        _NC = _split_drain_waits(_build_nc())
    return _NC


def _make_in_maps(query, context, Wq, Wk, Wv, Wo, bo):
    zeros_res = np.zeros((C, L), np.float32)
    in_maps = []
    for c in range(NCORES):
        b, hf = c // 2, c % 2
        rows = slice(hf * CS, (hf + 1) * CS)
        in_maps.append(
            {
                "q8": query[b].astype(F8),
                "ctx8": context[b].astype(F8),
                "wqT": np.ascontiguousarray(Wq[rows].T).astype(F8),
                "wkT": np.ascontiguousarray(Wk[rows].T).astype(F8),
                "wvT": np.ascontiguousarray(Wv[rows].T).astype(F8),
                "woT": np.ascontiguousarray(Wo[:, rows].T).astype(F8),
                "resid": (query[b] + bo[:, None]).astype(np.float32)
                if hf == 0
                else zeros_res,
            }
        )
    return in_maps


def _gather(results):
    out = np.empty((B, C, L), np.float32)
    for b in range(B):
        out[b] = results[2 * b]["out"] + results[2 * b + 1]["out"]
    return out


def kernel(query, context, Wq, Wk, Wv, Wo, bo, heads):
    query = np.asarray(query, dtype=np.float32)
    context = np.asarray(context, dtype=np.float32)
    Wq = np.asarray(Wq, dtype=np.float32)
    Wk = np.asarray(Wk, dtype=np.float32)
    Wv = np.asarray(Wv, dtype=np.float32)
    Wo = np.asarray(Wo, dtype=np.float32)
    bo = np.asarray(bo, dtype=np.float32)
    assert int(heads) == H
    assert query.shape == (B, C, L) and context.shape == (B, C, L)

    nc = _get_nc()
    in_maps = _make_in_maps(query, context, Wq, Wk, Wv, Wo, bo)
    res = run_bass_kernel_spmd(nc, in_maps, list(range(NCORES))).results
    return _gather(res)


# revision 14
# speedup vs baseline: 1.5027x; 1.2509x over previous
"""CrossAttention1D Trainium2 kernel (fp8 DoubleRow edition).

Problem: B=4, C=1024, L=2048, H=16 heads (D=64). LKV == LQ so the
reference's linear interpolation is the identity and is skipped.

Sharding (8 cores): data-parallel over batch (4) x tensor-parallel over
heads (2 halves of 8 heads). Core c handles batch c//2, heads
(c%2)*8 .. (c%2)*8+8. Each core computes its half of Q/K/V projections
(512 of 1024 channels), attention for its 8 heads, and a partial output
projection Wo[:, shard] @ O (+ residual/bias on even cores). The host
sums the two partials per batch.

Device dataflow per core:
  All projections + AV + Wo run as fp8e4m3 DoubleRow matmuls (2 k-tiles
  of 128 packed per instruction, 2x PE throughput); QK^T stays bf16 with
  k=64 row-group pairing (both heads of a pair concurrently on the PE).
  Softmax uses exp(S/8 - 3) (shift-invariant; keeps P inside fp8 range)
  written by ACT directly to fp8, with a ones-column in the V stationary
  accumulating the denominator row during AV. O is normalized by
  reciprocal_approx_fast + PE broadcast, stored fp8 for the Wo DoubleRow.
"""

import json

import numpy as np
import ml_dtypes

import concourse.bass as bass
import concourse.mybir as mybir
import concourse.tile as tile
from concourse.bass_utils import run_bass_kernel_spmd

BF16 = ml_dtypes.bfloat16
F8 = ml_dtypes.float8_e4m3

B, C, L, H, D = 4, 1024, 2048, 16, 64
CS = C // 2          # channel shard per core (512)
HPC = H // 2         # heads per core (8)
NCORES = 8
SCALE = 1.0 / np.sqrt(D)  # 0.125
EXP_BIAS = -3.0      # exp(S*SCALE + EXP_BIAS): softmax-invariant shift
VP = 80              # per-head stride in V_sb (65 used, 16B-aligned)

_DT = mybir.dt

_MAX_WAITS = 1


def _split_drain_waits(nc):
    """Hoist excess per-instruction sync-waits onto preceding NoOps.

    This toolchain's walrus codegen rejects instructions carrying more
    than one sync wait ("Too many sync wait commands"). Hoisting a wait
    onto a NoOp immediately before the instruction on the same engine is
    semantics-preserving (engines execute their stream in order).
    """
    j = json.loads(nc.to_json_bytes())
    n_hoisted = 0
    for fn in j["functions"]:
        for bb in fn["blocks"]:
            out = []
            for inst in bb["instructions"]:
                si = inst.get("sync_info")
                ow = (si or {}).get("on_wait") or []
                if len(ow) > _MAX_WAITS:
                    n_hoisted += 1
                    for i, w in enumerate(ow[: -_MAX_WAITS]):
                        out.append(
                            {
                                "engine": inst["engine"],
                                "ins": [],
                                "outs": [],
                                "name": f"{inst['name']}_hw{i}",
                                "opcode": "NoOp",
                                "debug": inst.get("debug"),
                                "sync_info": {"on_update": [], "on_wait": [w]},
                            }
                        )
                    si["on_wait"] = ow[-_MAX_WAITS:]
                out.append(inst)
            bb["instructions"] = out
    patched = json.dumps(j).encode()
    nc.to_json_bytes = lambda: patched
    return nc


def _build_nc():
    nc = bass.Bass()
    dt = _DT
    bf = dt.bfloat16
    f8 = dt.float8e4
    f32 = dt.float32
    DR = mybir.MatmulPerfMode.DoubleRow

    q_d = nc.declare_dram_parameter("q8", [C, L], f8, isOutput=False)
    ctx_d = nc.declare_dram_parameter("ctx8", [C, L], f8, isOutput=False)
    wq_d = nc.declare_dram_parameter("wqT", [C, CS], f8, isOutput=False)
    wk_d = nc.declare_dram_parameter("wkT", [C, CS], f8, isOutput=False)
    wv_d = nc.declare_dram_parameter("wvT", [C, CS], f8, isOutput=False)
    wo_d = nc.declare_dram_parameter("woT", [CS, C], f8, isOutput=False)
    res_d = nc.declare_dram_parameter("resid", [C, L], f32, isOutput=False)
    selm_d = nc.declare_dram_parameter(
        "selm", [HPC, HPC * 64], bf, isOutput=False
    )
    out_d = nc.declare_dram_parameter("out", [C, L], f32, isOutput=True)

    KT = C // 128        # 8 contraction tiles for projections
    KP = KT // 2         # 4 DoubleRow k-pairs
    CT = CS // 128       # 4 channel tiles of the shard
    CP = CT // 2         # 2 DoubleRow k-pairs for Wo
    LT = L // 512        # 4 L-tiles of 512
    JT = L // 128        # 16 j-tiles of 128
    JP = JT // 2         # 8 j-pairs (DoubleRow AV)

    with tile.TileContext(nc) as tc:
        with (
            tc.tile_pool(name="const", bufs=1) as cp,
            tc.tile_pool(name="pwork", bufs=3) as pwork,
            tc.tile_pool(name="norm", bufs=2) as normp,
            tc.tile_pool(name="io", bufs=3) as iop,
            tc.tile_pool(name="psum", bufs=3, space="PSUM") as psp,
        ):
            # ---- resident SBUF slabs
            q_sb = cp.tile([128, KT, L], f8)       # query, c_in on partitions
            c_sb = cp.tile([128, KT, L], f8)       # context
            wq_sb = cp.tile([128, KT, CS], f8)
            wk_sb = cp.tile([128, KT, CS], f8)
            wv_sb = cp.tile([128, KT, CS], f8)
            wo_sb = cp.tile([128, CT, C], f8)
            Q_sb = cp.tile([128, CT, L], bf)       # projected Q (bf16)
            K_sb = cp.tile([128, CT, L], bf)
            V_sb = cp.tile([128, JT, HPC * VP], f8)  # V^T + ones cols, padded
            O_sb = cp.tile([128, CT, L], f8)       # normalized attn output

            qr = q_d.rearrange("(k p) m -> p k m", p=128)
            cr = ctx_d.rearrange("(k p) m -> p k m", p=128)
            wqr = wq_d.rearrange("(k p) m -> p k m", p=128)
            wkr = wk_d.rearrange("(k p) m -> p k m", p=128)
            wvr = wv_d.rearrange("(k p) m -> p k m", p=128)
            wor = wo_d.rearrange("(k p) m -> p k m", p=128)
            for kt in range(KT):
                nc.sync.dma_start(q_sb[:, kt, :], qr[:, kt, :])
                nc.sync.dma_start(c_sb[:, kt, :], cr[:, kt, :])
                nc.sync.dma_start(wq_sb[:, kt, :], wqr[:, kt, :])
                nc.sync.dma_start(wk_sb[:, kt, :], wkr[:, kt, :])
                nc.sync.dma_start(wv_sb[:, kt, :], wvr[:, kt, :])
            for kt in range(CT):
                nc.sync.dma_start(wo_sb[:, kt, :], wor[:, kt, :])

            # ones columns for the AV denominator rows
            v_view = V_sb.rearrange("p j (h e) -> p j h e", e=VP)
            for jt in range(JT):
                nc.vector.memset(v_view[:, jt, :, D : D + 1], 1.0)
            # selm[p, h*64:(h+1)*64] = 1 iff p == h: lhsT selector that
            # broadcasts row h of an [8, N] rhs onto 64 output partitions.
            selm = cp.tile([HPC, HPC * 64], bf)
            nc.sync.dma_start(selm, selm_d[:, :])
            bias_sb = cp.tile([128, 1], f32)
            nc.vector.memset(bias_sb, EXP_BIAS)
            zbias_sb = cp.tile([128, 1], f32)
            nc.vector.memset(zbias_sb, 0.0)

            # ---- projections (fp8 DoubleRow, contraction 256/matmul)
            # K first, then V, then Q i-tile 0: attention can start early.
            def emit_proj_qk(dst, w_sb, src, ct, lt):
                ls = slice(lt * 512, (lt + 1) * 512)
                p = psp.tile([128, 512], f32, tag="s")
                for kp in range(KP):
                    nc.tensor.matmul(
                        p,
                        lhsT=w_sb[:, 2 * kp : 2 * kp + 2, ct * 128 : (ct + 1) * 128],
                        rhs=src[:, 2 * kp : 2 * kp + 2, ls],
                        start=(kp == 0),
                        stop=(kp == KP - 1),
                        perf_mode=DR,
                    )
                nc.vector.tensor_copy(dst[:, ct, ls], p)

            def emit_proj_v(jt):
                pv = psp.tile([128, 512], f32, tag="s")
                for kp in range(KP):
                    nc.tensor.matmul(
                        pv,
                        lhsT=c_sb[:, 2 * kp : 2 * kp + 2, jt * 128 : (jt + 1) * 128],
                        rhs=wv_sb[:, 2 * kp : 2 * kp + 2, :],
                        start=(kp == 0),
                        stop=(kp == KP - 1),
                        perf_mode=DR,
                    )
                nc.vector.tensor_copy(
                    v_view[:, jt, :, 0:D],
                    pv.rearrange("p (h d) -> p h d", d=D),
                )

            for ct in range(CT):
                for lt in range(LT):
                    emit_proj_qk(K_sb, wk_sb, c_sb, ct, lt)
            for jt in range(JT):
                emit_proj_v(jt)
            for ct in range(CT):
                emit_proj_qk(Q_sb, wq_sb, q_sb, ct, 0)

            # ---- attention + output projection, per i-tile epoch.
            # QK (bf16, k=64 row-group paired) -> exp to fp8 -> AV DoubleRow.
            # Wo-projection matmuls for the previous i-tile and the
            # remaining Q projections are interleaved as PE filler.
            Exp = mybir.ActivationFunctionType.Exp
            DEPTH = 2

            def emit_wo(mt, it_prev):
                psl = slice(it_prev * 512, (it_prev + 1) * 512)
                po = psp.tile([128, 512], f32, tag="s")
                for kp in range(CP):
                    nc.tensor.matmul(
                        po,
                        lhsT=wo_sb[:, 2 * kp : 2 * kp + 2, mt * 128 : (mt + 1) * 128],
                        rhs=O_sb[:, 2 * kp : 2 * kp + 2, psl],
                        start=(kp == 0),
                        stop=(kp == CP - 1),
                        perf_mode=DR,
                    )
                rt = iop.tile([128, 512], f32, tag="res")
                nc.sync.dma_start(rt, res_d[mt * 128 : (mt + 1) * 128, psl])
                ot = iop.tile([128, 512], f32, tag="out")
                nc.vector.tensor_add(ot, po, rt)
                nc.sync.dma_start(out_d[mt * 128 : (mt + 1) * 128, psl], ot)

            Ln = mybir.ActivationFunctionType.Ln

            for it in range(LT):
                isl = slice(it * 512, (it + 1) * 512)
                dn = normp.tile([HPC, 512], bf, tag="dn", bufs=2)
                oraw = [
                    normp.tile([D + 1, 512], bf, tag=f"oraw{h}",
                               name=f"oraw{h}", bufs=2)
                    for h in range(HPC)
                ]
                filler = []
                if it > 0:
                    # Q projection for this epoch's successor is already done
                    # (lt=it emitted below); Wo for previous epoch.
                    filler = [("wo", mt) for mt in range(C // 128)]
                if it < LT - 1:
                    filler += [("q", ct) for ct in range(CT)]
                for tp in range(CT):  # head pair (2*tp, 2*tp+1)
                    pOa = psp.tile([D + 1, 512], f32, tag="acc", bufs=2)
                    pOb = psp.tile([D + 1, 512], f32, tag="acc", bufs=2)
                    pend = []
                    for jp in range(JP + DEPTH):
                        if jp < JP:
                            Pab = pwork.tile([128, 2, 1024], f8, tag="p")
                            for t in range(2):
                                jt = 2 * jp + t
                                js = slice(jt * 128, (jt + 1) * 128)
                                pS = psp.tile([128, 1024], f32, tag="s")
                                nc.tensor.matmul(
                                    pS[:, 0:512],
                                    lhsT=K_sb[0:64, tp, js],
                                    rhs=Q_sb[0:64, tp, isl],
                                    start=True,
                                    stop=True,
                                )
                                nc.tensor.matmul(
                                    pS[:, 512:1024],
                                    lhsT=K_sb[64:128, tp, js],
                                    rhs=Q_sb[64:128, tp, isl],
                                    start=True,
                                    stop=True,
                                )
                                nc.scalar.activation(
                                    Pab[:, t, :], pS, Exp,
                                    bias=bias_sb[:, :], scale=SCALE,
                                )
                            pend.append((Pab, jp))
                        if len(pend) > (DEPTH if jp < JP else 0):
                            Pab_r, qjp = pend.pop(0)
                            ha, hb = 2 * tp, 2 * tp + 1
                            nc.tensor.matmul(
                                pOa,
                                lhsT=V_sb[:, 2 * qjp : 2 * qjp + 2,
                                          ha * VP : ha * VP + D + 1],
                                rhs=Pab_r[:, 0:2, 0:512],
                                start=(qjp == 0),
                                stop=(qjp == JP - 1),
                                perf_mode=DR,
                            )
                            nc.tensor.matmul(
                                pOb,
                                lhsT=V_sb[:, 2 * qjp : 2 * qjp + 2,
                                          hb * VP : hb * VP + D + 1],
                                rhs=Pab_r[:, 0:2, 512:1024],
                                start=(qjp == 0),
                                stop=(qjp == JP - 1),
                                perf_mode=DR,
                            )
                        if jp % 2 == 1 and filler:
                            kind, arg = filler.pop(0)
                            if kind == "wo":
                                emit_wo(arg, it - 1)
                            else:
                                emit_proj_qk(Q_sb, wq_sb, q_sb, arg, it + 1)

                    # stage both heads' raw O (+denominator row) to SBUF,
                    # releasing the PSUM accumulators quickly; the actual
                    # normalization is batched at epoch end.
                    for hh, pO in ((2 * tp, pOa), (2 * tp + 1, pOb)):
                        nc.vector.tensor_copy(oraw[hh], pO)
                        nc.sync.dma_start(
                            dn[hh : hh + 1, :], oraw[hh][D : D + 1, :]
                        )
                for kind, arg in filler:
                    if kind == "wo":
                        emit_wo(arg, it - 1)
                    else:
                        emit_proj_qk(Q_sb, wq_sb, q_sb, arg, it + 1)

                # batched normalization: 1/denom = exp(-ln(denom)) on ACT,
                # then per-head PE broadcast + fused multiply to fp8 O_sb.
                lnd = normp.tile([HPC, 512], f32, tag="lnd", bufs=2)
                nc.scalar.activation(lnd, dn, Ln, bias=zbias_sb[:HPC, :])
                recb = normp.tile([HPC, 512], bf, tag="recb", bufs=2)
                nc.scalar.activation(
                    recb, lnd, Exp, bias=zbias_sb[:HPC, :], scale=-1.0
                )
                for hh in range(HPC):
                    rb_ps = psp.tile([64, 512], f32, tag="s")
                    nc.tensor.matmul(
                        rb_ps,
                        lhsT=selm[:, hh * 64 : (hh + 1) * 64],
                        rhs=recb,
                        start=True, stop=True,
                    )
                    otmp = normp.tile([64, 512], f8, tag="otmp")
                    nc.vector.tensor_mul(otmp, oraw[hh][0:D, :], rb_ps)
                    poff = (hh % 2) * 64
                    nc.sync.dma_start(
                        O_sb[poff : poff + 64, hh // 2, isl], otmp
                    )
            for mt in range(C // 128):
                emit_wo(mt, LT - 1)
    return nc


_NC = None


def _get_nc():
    global _NC
    if _NC is None:
        _NC = _split_drain_waits(_build_nc())
    return _NC


def _make_in_maps(query, context, Wq, Wk, Wv, Wo, bo):
    zeros_res = np.zeros((C, L), np.float32)
    selm = np.zeros((HPC, HPC * 64), dtype=BF16)
    for h in range(HPC):
        selm[h, h * 64 : (h + 1) * 64] = 1.0
    in_maps = []
    for c in range(NCORES):
        b, hf = c // 2, c % 2
        rows = slice(hf * CS, (hf + 1) * CS)
        in_maps.append(
            {
                "q8": query[b].astype(F8),
                "ctx8": context[b].astype(F8),
                "wqT": np.ascontiguousarray(Wq[rows].T).astype(F8),
                "wkT": np.ascontiguousarray(Wk[rows].T).astype(F8),
                "wvT": np.ascontiguousarray(Wv[rows].T).astype(F8),
                "woT": np.ascontiguousarray(Wo[:, rows].T).astype(F8),
                "resid": (query[b] + bo[:, None]).astype(np.float32)
                if hf == 0
                else zeros_res,
                "selm": selm,
            }
        )
    return in_maps


def _gather(results):
    out = np.empty((B, C, L), np.float32)
    for b in range(B):
        out[b] = results[2 * b]["out"] + results[2 * b + 1]["out"]
    return out


def kernel(query, context, Wq, Wk, Wv, Wo, bo, heads):
    query = np.asarray(query, dtype=np.float32)
    context = np.asarray(context, dtype=np.float32)
    Wq = np.asarray(Wq, dtype=np.float32)
    Wk = np.asarray(Wk, dtype=np.float32)
    Wv = np.asarray(Wv, dtype=np.float32)
    Wo = np.asarray(Wo, dtype=np.float32)
    bo = np.asarray(bo, dtype=np.float32)
    assert int(heads) == H
    assert query.shape == (B, C, L) and context.shape == (B, C, L)

    nc = _get_nc()
    in_maps = _make_in_maps(query, context, Wq, Wk, Wv, Wo, bo)
    res = run_bass_kernel_spmd(nc, in_maps, list(range(NCORES))).results
    return _gather(res)


# revision 20
# speedup vs baseline: 1.6375x; 1.0897x over previous
"""CrossAttention1D Trainium2 kernel (fp8 DoubleRow edition).

Problem: B=4, C=1024, L=2048, H=16 heads (D=64). LKV == LQ so the
reference's linear interpolation is the identity and is skipped.

Sharding (8 cores): data-parallel over batch (4) x tensor-parallel over
heads (2 halves of 8 heads). Core c handles batch c//2, heads
(c%2)*8 .. (c%2)*8+8. Each core computes its half of Q/K/V projections
(512 of 1024 channels), attention for its 8 heads, and a partial output
projection Wo[:, shard] @ O (+ residual/bias on even cores). The host
sums the two partials per batch.

Device dataflow per core:
  All projections + AV + Wo run as fp8e4m3 DoubleRow matmuls (2 k-tiles
  of 128 packed per instruction, 2x PE throughput); QK^T stays bf16 with
  k=64 row-group pairing (both heads of a pair concurrently on the PE).
  Softmax uses exp(S/8 - 3) (shift-invariant; keeps P inside fp8 range)
  written by ACT directly to fp8, with a ones-column in the V stationary
  accumulating the denominator row during AV. Per head pair, raw O is
  staged to SBUF (releasing PSUM accumulators), the denominator row is
  inverted on DVE, and a selector-matmul broadcast + fused multiply
  produce fp8 O for the Wo DoubleRow. All non-attention work (remaining
  projections, normalization, Wo) is interleaved into the attention jp
  loop as slot-gated PE filler so the ACT engine (the roofline at ~1us
  per 128x1024 exp tile) never waits.
"""

import json

import numpy as np
import ml_dtypes

import concourse.bass as bass
import concourse.mybir as mybir
import concourse.tile as tile
from concourse.bass_utils import run_bass_kernel_spmd

BF16 = ml_dtypes.bfloat16
F8 = ml_dtypes.float8_e4m3

B, C, L, H, D = 4, 1024, 2048, 16, 64
CS = C // 2          # channel shard per core (512)
HPC = H // 2         # heads per core (8)
NCORES = 8
SCALE = 1.0 / np.sqrt(D)  # 0.125
EXP_BIAS = -3.0      # exp(S*SCALE + EXP_BIAS): softmax-invariant shift
VP = 80              # per-head stride in V_sb (65 used, 16B-aligned)

_DT = mybir.dt

_MAX_WAITS = 1


def _split_drain_waits(nc):
    """Hoist excess per-instruction sync-waits onto preceding NoOps.

    This toolchain's walrus codegen rejects instructions carrying more
    than one sync wait ("Too many sync wait commands"). Hoisting a wait
    onto a NoOp immediately before the instruction on the same engine is
    semantics-preserving (engines execute their stream in order).
    """
    j = json.loads(nc.to_json_bytes())
    n_hoisted = 0
    for fn in j["functions"]:
        for bb in fn["blocks"]:
            out = []
            for inst in bb["instructions"]:
                si = inst.get("sync_info")
                ow = (si or {}).get("on_wait") or []
                if len(ow) > _MAX_WAITS:
                    n_hoisted += 1
                    for i, w in enumerate(ow[: -_MAX_WAITS]):
                        out.append(
                            {
                                "engine": inst["engine"],
                                "ins": [],
                                "outs": [],
                                "name": f"{inst['name']}_hw{i}",
                                "opcode": "NoOp",
                                "debug": inst.get("debug"),
                                "sync_info": {"on_update": [], "on_wait": [w]},
                            }
                        )
                    si["on_wait"] = ow[-_MAX_WAITS:]
                out.append(inst)
            bb["instructions"] = out
    patched = json.dumps(j).encode()
    nc.to_json_bytes = lambda: patched
    return nc


def _build_nc():
    nc = bass.Bass()
    dt = _DT
    bf = dt.bfloat16
    f8 = dt.float8e4
    f32 = dt.float32
    DR = mybir.MatmulPerfMode.DoubleRow

    q_d = nc.declare_dram_parameter("q8", [C, L], f8, isOutput=False)
    ctx_d = nc.declare_dram_parameter("ctx8", [C, L], f8, isOutput=False)
    wq_d = nc.declare_dram_parameter("wqT", [C, CS], f8, isOutput=False)
    wk_d = nc.declare_dram_parameter("wkT", [C, CS], f8, isOutput=False)
    wv_d = nc.declare_dram_parameter("wvT", [C, CS], f8, isOutput=False)
    wo_d = nc.declare_dram_parameter("woT", [CS, C], f8, isOutput=False)
    res_d = nc.declare_dram_parameter("resid", [C, L], f32, isOutput=False)
    selm_d = nc.declare_dram_parameter(
        "selm", [HPC, HPC * 64], bf, isOutput=False
    )
    out_d = nc.declare_dram_parameter("out", [C, L], f32, isOutput=True)

    KT = C // 128        # 8 contraction tiles for projections
    KP = KT // 2         # 4 DoubleRow k-pairs
    CT = CS // 128       # 4 channel tiles of the shard
    CP = CT // 2         # 2 DoubleRow k-pairs for Wo
    LT = L // 512        # 4 L-tiles of 512
    JT = L // 128        # 16 j-tiles of 128
    JP = JT // 2         # 8 j-pairs (DoubleRow AV)

    with tile.TileContext(nc) as tc:
        with (
            tc.tile_pool(name="const", bufs=1) as cp,
            tc.tile_pool(name="pwork", bufs=3) as pwork,
            tc.tile_pool(name="norm", bufs=2) as normp,
            tc.tile_pool(name="io", bufs=3) as iop,
            tc.tile_pool(name="psum", bufs=3, space="PSUM") as psp,
        ):
            # ---- resident SBUF slabs
            q_sb = cp.tile([128, KT, L], f8)       # query, c_in on partitions
            c_sb = cp.tile([128, KT, L], f8)       # context
            wq_sb = cp.tile([128, KT, CS], f8)
            wk_sb = cp.tile([128, KT, CS], f8)
            wv_sb = cp.tile([128, KT, CS], f8)
            wo_sb = cp.tile([128, CT, C], f8)
            Q_sb = cp.tile([128, CT, L], bf)       # projected Q (bf16)
            K_sb = cp.tile([128, CT, L], bf)
            V_sb = cp.tile([128, JT, HPC * VP], f8)  # V^T + ones cols, padded
            O_sb = cp.tile([128, CT, L], f8)       # normalized attn output

            qr = q_d.rearrange("(k p) m -> p k m", p=128)
            cr = ctx_d.rearrange("(k p) m -> p k m", p=128)
            wqr = wq_d.rearrange("(k p) m -> p k m", p=128)
            wkr = wk_d.rearrange("(k p) m -> p k m", p=128)
            wvr = wv_d.rearrange("(k p) m -> p k m", p=128)
            wor = wo_d.rearrange("(k p) m -> p k m", p=128)
            for kt in range(KT):
                nc.sync.dma_start(c_sb[:, kt, :], cr[:, kt, :])
                nc.sync.dma_start(wk_sb[:, kt, :], wkr[:, kt, :])
                nc.sync.dma_start(wv_sb[:, kt, :], wvr[:, kt, :])
                nc.sync.dma_start(q_sb[:, kt, :], qr[:, kt, :])
                nc.sync.dma_start(wq_sb[:, kt, :], wqr[:, kt, :])
            for kt in range(CT):
                nc.sync.dma_start(wo_sb[:, kt, :], wor[:, kt, :])

            # ones columns for the AV denominator rows
            v_view = V_sb.rearrange("p j (h e) -> p j h e", e=VP)
            for jt in range(JT):
                nc.vector.memset(v_view[:, jt, :, D : D + 1], 1.0)
            # selm[p, h*64:(h+1)*64] = 1 iff p == h: lhsT selector that
            # broadcasts row h of an [HPC, N] rhs onto 64 output partitions.
            selm = cp.tile([HPC, HPC * 64], bf)
            nc.sync.dma_start(selm, selm_d[:, :])
            bias_sb = cp.tile([128, 1], f32)
            nc.vector.memset(bias_sb, EXP_BIAS)

            # ---- emission helpers
            def emit_proj_qk(dst, w_sb, src, ct, lt):
                ls = slice(lt * 512, (lt + 1) * 512)
                p = psp.tile([128, 512], f32, tag="s")
                for kp in range(KP):
                    nc.tensor.matmul(
                        p,
                        lhsT=w_sb[:, 2 * kp : 2 * kp + 2, ct * 128 : (ct + 1) * 128],
                        rhs=src[:, 2 * kp : 2 * kp + 2, ls],
                        start=(kp == 0),
                        stop=(kp == KP - 1),
                        perf_mode=DR,
                    )
                nc.vector.tensor_copy(dst[:, ct, ls], p)

            def emit_proj_v(jt):
                pv = psp.tile([128, 512], f32, tag="s")
                for kp in range(KP):
                    nc.tensor.matmul(
                        pv,
                        lhsT=c_sb[:, 2 * kp : 2 * kp + 2, jt * 128 : (jt + 1) * 128],
                        rhs=wv_sb[:, 2 * kp : 2 * kp + 2, :],
                        start=(kp == 0),
                        stop=(kp == KP - 1),
                        perf_mode=DR,
                    )
                nc.vector.tensor_copy(
                    v_view[:, jt, :, 0:D],
                    pv.rearrange("p (h d) -> p h d", d=D),
                )

            def emit_wo(state, mt):
                it_prev = state["it"]
                psl = slice(it_prev * 512, (it_prev + 1) * 512)
                po = psp.tile([128, 512], f32, tag="s")
                for kp in range(CP):
                    nc.tensor.matmul(
                        po,
                        lhsT=wo_sb[:, 2 * kp : 2 * kp + 2, mt * 128 : (mt + 1) * 128],
                        rhs=O_sb[:, 2 * kp : 2 * kp + 2, psl],
                        start=(kp == 0),
                        stop=(kp == CP - 1),
                        perf_mode=DR,
                    )
                ot = iop.tile([128, 512], f32, tag="out")
                nc.vector.tensor_add(ot, po, state["rt"][mt])
                nc.sync.dma_start(out_d[mt * 128 : (mt + 1) * 128, psl], ot)

            def emit_norm(state, hh):
                # PE broadcast of 1/denom row + fused multiply to fp8 O_sb
                it_prev, row = state["it"], hh % 2
                rb_ps = psp.tile([64, 512], f32, tag="s")
                nc.tensor.matmul(
                    rb_ps,
                    lhsT=selm[0:2, row * 64 : (row + 1) * 64],
                    rhs=state["recb"],
                    start=True, stop=True,
                )
                otmp = normp.tile([64, 512], f8, tag="otmp")
                nc.vector.tensor_mul(otmp, state["oraw"][hh][0:D, :], rb_ps)
                poff = (hh % 2) * 64
                psl = slice(it_prev * 512, (it_prev + 1) * 512)
                nc.sync.dma_start(O_sb[poff : poff + 64, hh // 2, psl], otmp)

            # ---- upfront projections: K (all), V j-tiles 0..3, Q i-tile 0.
            # The rest streams in as PE filler inside the attention loop.
            for ct in range(CT):
                for lt in range(LT):
                    emit_proj_qk(K_sb, wk_sb, c_sb, ct, lt)
            for jt in range(4):
                emit_proj_v(jt)
            for ct in range(CT):
                emit_proj_qk(Q_sb, wq_sb, q_sb, ct, 0)

            # ---- attention epochs with slot-gated filler
            Exp = mybir.ActivationFunctionType.Exp
            DEPTH = 2
            SLOTS = JP + DEPTH  # per head pair

            pending = []  # filler carried into the next epoch

            for it in range(LT):
                isl = slice(it * 512, (it + 1) * 512)
                oraw = [
                    normp.tile([D + 1, 512], bf, tag=f"oraw{h}",
                               name=f"oraw{h}", bufs=2)
                    for h in range(HPC)
                ]
                filler = pending
                pending = []
                if it == 0:
                    filler += [(0, "v", None, jt) for jt in range(4, JT)]
                if it < LT - 1:
                    filler += [(0, "q", None, ct) for ct in range(CT)]

                def do_filler(slot, budget=2):
                    while budget > 0 and filler and filler[0][0] <= slot:
                        _, kind, state, arg = filler.pop(0)
                        if kind == "norm":
                            emit_norm(state, arg)
                        elif kind == "wo":
                            emit_wo(state, arg)
                        elif kind == "v":
                            emit_proj_v(arg)
                        else:
                            emit_proj_qk(Q_sb, wq_sb, q_sb, arg, it + 1)
                        budget -= 1 if kind == "v" else 2

                for tp in range(CT):  # head pair (2*tp, 2*tp+1)
                    pOa = psp.tile([D + 1, 512], f32, tag="acc", bufs=2)
                    pOb = psp.tile([D + 1, 512], f32, tag="acc", bufs=2)
                    pend = []
                    for jp in range(SLOTS):
                        if jp < JP:
                            Pab = pwork.tile([128, 2, 1024], f8, tag="p")
                            for t in range(2):
                                jt = 2 * jp + t
                                js = slice(jt * 128, (jt + 1) * 128)
                                pS = psp.tile([128, 1024], f32, tag="s")
                                nc.tensor.matmul(
                                    pS[:, 0:512],
                                    lhsT=K_sb[0:64, tp, js],
                                    rhs=Q_sb[0:64, tp, isl],
                                    start=True,
                                    stop=True,
                                )
                                nc.tensor.matmul(
                                    pS[:, 512:1024],
                                    lhsT=K_sb[64:128, tp, js],
                                    rhs=Q_sb[64:128, tp, isl],
                                    start=True,
                                    stop=True,
                                )
                                nc.scalar.activation(
                                    Pab[:, t, :], pS, Exp,
                                    bias=bias_sb[:, :], scale=SCALE,
                                )
                            pend.append((Pab, jp))
                        if len(pend) > (DEPTH if jp < JP else 0):
                            Pab_r, qjp = pend.pop(0)
                            ha, hb = 2 * tp, 2 * tp + 1
                            nc.tensor.matmul(
                                pOa,
                                lhsT=V_sb[:, 2 * qjp : 2 * qjp + 2,
                                          ha * VP : ha * VP + D + 1],
                                rhs=Pab_r[:, 0:2, 0:512],
                                start=(qjp == 0),
                                stop=(qjp == JP - 1),
                                perf_mode=DR,
                            )
                            nc.tensor.matmul(
                                pOb,
                                lhsT=V_sb[:, 2 * qjp : 2 * qjp + 2,
                                          hb * VP : hb * VP + D + 1],
                                rhs=Pab_r[:, 0:2, 512:1024],
                                start=(qjp == 0),
                                stop=(qjp == JP - 1),
                                perf_mode=DR,
                            )
                        do_filler(tp * SLOTS + jp)

                    # stage both heads' raw O (+denominator row) to SBUF,
                    # releasing the PSUM accumulators; invert the pair's
                    # denominators on DVE; normalization multiplies join the
                    # filler stream ~1.5 pairs later (recb latency ~5us).
                    dn2 = normp.tile([2, 512], bf, tag="dn", bufs=2)
                    for i2, pO in ((0, pOa), (1, pOb)):
                        nc.vector.tensor_copy(oraw[2 * tp + i2], pO)
                        nc.sync.dma_start(
                            dn2[i2 : i2 + 1, :],
                            oraw[2 * tp + i2][D : D + 1, :],
                        )
                    recf = normp.tile([2, 512], f32, tag="recf", bufs=2)
                    nc.vector.reciprocal(recf, dn2)
                    recb = normp.tile([2, 512], bf, tag="recb", bufs=4)
                    nc.vector.tensor_copy(recb, recf)
                    state = {"recb": recb, "oraw": oraw, "it": it}
                    filler += [
                        ((tp + 1) * SLOTS + 5, "norm", state, 2 * tp + i2)
                        for i2 in range(2)
                    ]

                # leftovers (pair-3 norms and any stragglers) carry over,
                # keeping their slot phase relative to the new epoch
                for f in filler:
                    pending.append((max(0, f[0] - CT * SLOTS), f[1], f[2], f[3]))
                # Wo for this epoch runs in the next one; prefetch residuals
                # now so the adds never wait on DMA.
                rt = [
                    iop.tile([128, 512], f32, tag=f"res{mt}",
                             name=f"res{mt}", bufs=2)
                    for mt in range(C // 128)
                ]
                psl = isl
                for mt in range(C // 128):
                    nc.sync.dma_start(
                        rt[mt], res_d[mt * 128 : (mt + 1) * 128, psl]
                    )
                wstate = {"rt": rt, "it": it}
                pending += [(8, "wo", wstate, mt) for mt in range(C // 128)]

            # final flush: pair-3 norms of the last epoch + its Wo batch
            filler = pending
            while filler:
                _, kind, state, arg = filler.pop(0)
                if kind == "norm":
                    emit_norm(state, arg)
                else:
                    emit_wo(state, arg)
    return nc


_NC = None


def _get_nc():
    global _NC
    if _NC is None:
        _NC = _split_drain_waits(_build_nc())
    return _NC


def _make_in_maps(query, context, Wq, Wk, Wv, Wo, bo):
    zeros_res = np.zeros((C, L), np.float32)
    selm = np.zeros((HPC, HPC * 64), dtype=BF16)
    for h in range(HPC):
        selm[h, h * 64 : (h + 1) * 64] = 1.0
    in_maps = []
    for c in range(NCORES):
        b, hf = c // 2, c % 2
        rows = slice(hf * CS, (hf + 1) * CS)
        in_maps.append(
            {
                "q8": query[b].astype(F8),
                "ctx8": context[b].astype(F8),
                "wqT": np.ascontiguousarray(Wq[rows].T).astype(F8),
                "wkT": np.ascontiguousarray(Wk[rows].T).astype(F8),
                "wvT": np.ascontiguousarray(Wv[rows].T).astype(F8),
                "woT": np.ascontiguousarray(Wo[:, rows].T).astype(F8),
                "resid": (query[b] + bo[:, None]).astype(np.float32)
                if hf == 0
                else zeros_res,
                "selm": selm,
            }
        )
    return in_maps


def _gather(results):
    out = np.empty((B, C, L), np.float32)
    for b in range(B):
        out[b] = results[2 * b]["out"] + results[2 * b + 1]["out"]
    return out


def kernel(query, context, Wq, Wk, Wv, Wo, bo, heads):
    query = np.asarray(query, dtype=np.float32)
    context = np.asarray(context, dtype=np.float32)
    Wq = np.asarray(Wq, dtype=np.float32)
    Wk = np.asarray(Wk, dtype=np.float32)
    Wv = np.asarray(Wv, dtype=np.float32)
    Wo = np.asarray(Wo, dtype=np.float32)
    bo = np.asarray(bo, dtype=np.float32)
    assert int(heads) == H
    assert query.shape == (B, C, L) and context.shape == (B, C, L)

    nc = _get_nc()
    in_maps = _make_in_maps(query, context, Wq, Wk, Wv, Wo, bo)
    res = run_bass_kernel_spmd(nc, in_maps, list(range(NCORES))).results
    return _gather(res)
